# revision 1
# baseline (speedup 1.0000x reference)
# GNN NNConv (3-layer) + pairwise-L1 CBT kernel for TRN2, 8 NeuronCores.
#
# Strategy:
#  - Host: sort edges by dst, pad each node's edge list to L=TPN*128 slots,
#    shard nodes across 8 cores (64 nodes/core). Padded slots have
#    edge_attr-extended == 0 (incl. the bias-ones row) so relu(A_pad)=0
#    contributes nothing to segment sums. Mean division uses host-computed
#    1/max(deg,1).
#  - Device per layer l: A = eaX @ [We;be]  (PE, K=7, edges on partitions)
#    T = relu(A) * x[src]-broadcast         (DVE scalar_tensor_tensor, or
#                                            ACT relu + DVE mult, mixed)
#    U[node] = sum_{edges of node} T        (PE matmul, eyeband stationary,
#                                            col-split over two PSUM groups)
#    s = (reduce_i U) * invcnt + hT@[root;b] (small DVE/PE ops)
#    h = relu(s); AllGather h (bf16, padded rows) -> DRAM; next layer
#    gathers x[src] rows via dma_gather.
#  - CBT: replicate h3^T via K=1 matmul, diff, reduce-with-abs over k.
import sys
import numpy as np

for p in ("/opt/trn_rl_repo",):
    if p not in sys.path:
        sys.path.insert(0, p)

import ml_dtypes
import concourse.bass as bass
import concourse.mybir as mybir
from concourse import bacc
from concourse import tile
from concourse import library_config
from concourse.bass import AP

BF16 = mybir.dt.bfloat16
F32 = mybir.dt.float32
I16 = mybir.dt.int16
AF = mybir.ActivationFunctionType
OP = mybir.AluOpType
nbf16 = ml_dtypes.bfloat16

DIMS = [(1, 36), (36, 24), (24, 5)]
V = 6


class P:
    """Problem geometry parameters."""

    def __init__(self, N, n_cores, TPN, debug=False):
        self.debug = debug
        self.N = N
        self.NC = n_cores
        self.NPC = N // n_cores          # nodes per core
        self.TPN = TPN                   # 128-edge tiles per node
        self.L = TPN * 128               # padded edges per node
        self.TILES = self.NPC * TPN      # tiles per core
        self.EC = self.TILES * 128       # padded edges per core
        self.GW = 128                    # gather row width (bf16) = 256B
        self.XCH = 4 if self.TILES % 4 == 0 else 1   # gather chunks
        self.CT = self.TILES // self.XCH  # tiles per gather chunk
        # per-layer tile group sizes for the wide elementwise op
        self.G = []
        for (ci, co) in DIMS:
            F = ci * co
            g = 1
            for cand in (8, 4, 2):
                if (self.TILES % cand == 0 and self.CT % cand == 0
                        and cand * F <= 512):
                    g = cand
                    break
            self.G.append(g)


# ---------------------------------------------------------------- host prep


def host_prep(x, edge_attr, edge_index, weights, p: P):
    """weights: dict with We1,be1,root1,b1,... Returns per-core input maps."""
    N, NPC, L = p.N, p.NPC, p.L
    src = np.asarray(edge_index[0], dtype=np.int64)
    dst = np.asarray(edge_index[1], dtype=np.int64)
    E = src.shape[0]
    x = np.asarray(x, dtype=np.float32)
    ea = np.asarray(edge_attr, dtype=np.float32)

    order = np.argsort(dst, kind="stable")
    dst_sorted = dst[order]
    starts = np.searchsorted(dst_sorted, np.arange(N))
    ends = np.searchsorted(dst_sorted, np.arange(N) + 1)
    deg = (ends - starts).astype(np.int64)
    assert deg.max() <= L, f"max degree {deg.max()} > L={L}"

    # slot -> edge id (or -1)
    slot_edge = np.full((N, L), -1, dtype=np.int64)
    for n in range(N):
        ids = order[starts[n]:ends[n]]
        slot_edge[n, : ids.shape[0]] = ids

    in_maps = []
    # shared (same on all cores) constants
    shared = {}
    for li, (ci, co) in enumerate(DIMS, start=1):
        We = np.asarray(weights[f"We{li}"], dtype=np.float32)   # [V, ci*co]
        be = np.asarray(weights[f"be{li}"], dtype=np.float32)   # [ci*co]
        root = np.asarray(weights[f"root{li}"], dtype=np.float32)  # [ci, co]
        b = np.asarray(weights[f"b{li}"], dtype=np.float32)     # [co]
        # permute feature axis to o-major: f' = o*ci + i  (from f = i*co + o)
        perm = (np.arange(ci * co).reshape(ci, co).T).reshape(-1)  # f'[k] = old idx
        WeP = np.concatenate([We[:, perm], be[perm][None, :]], axis=0)  # [7, F]
        shared[f"wep{li}"] = WeP.astype(nbf16)
        if li == 1:
            shared[f"rootb{li}"] = np.concatenate([root, b[None, :]], axis=0).astype(
                np.float32
            )  # [2, co]
        else:
            shared[f"rootb{li}"] = root.astype(np.float32)       # [ci, co]
            shared[f"bvec{li}"] = b[None, :].astype(np.float32)  # [1, co]
    shared["onesseg"] = np.ones((128, 1), dtype=nbf16)
    eyeband = np.zeros((128, 2 * NPC - 1), dtype=nbf16)
    eyeband[:, NPC - 1] = 1.0
    shared["eyeband"] = eyeband
    shared["ones1"] = np.ones((1, NPC), dtype=np.float32)
    shared["ident"] = np.eye(128, dtype=np.float32)

    for c in range(p.NC):
        nodes = np.arange(c * NPC, (c + 1) * NPC)
        se = slot_edge[nodes].reshape(-1)          # [EC]
        real = se >= 0
        e_ids = np.where(real, se, 0)

        ea7 = np.zeros((7, p.EC), dtype=np.float32)
        ea7[:6, :] = np.where(real[None, :], ea[e_ids].T, 0.0)
        ea7[6, :] = np.where(real, 1.0, 0.0)

        s16 = np.where(real, src[e_ids], 0).astype(np.int16)   # [EC]
        src16 = np.zeros((128, p.EC // 16), dtype=np.int16)
        idx = np.arange(p.EC)
        for k in range(8):  # Q7 cores each read their own 16-partition block
            src16[16 * k + (idx % 16), idx // 16] = s16

        xg1 = np.where(real, x[s16.astype(np.int64), 0], 0.0).astype(np.float32)
        xg1T = np.zeros((128, p.TILES), dtype=nbf16)
        xg1T[idx % 128, idx // 128] = xg1.astype(nbf16)

        invc = (1.0 / np.maximum(deg[nodes], 1)).astype(np.float32)[:, None]
        xb1T = np.stack([x[nodes, 0], np.ones(NPC, np.float32)], axis=0).astype(
            np.float32
        )  # [2, NPC]

        m = {
            "ea7": ea7.astype(nbf16),
            "src16": src16,
            "xg1T": xg1T,
            "invcnt": invc,
            "xb1T": xb1T,
        }
        m.update(shared)
        in_maps.append(m)
    return in_maps


# ------------------------------------------------------------- device build


def declare_io(nc: bass.Bass, p: P):
    """Declare DRAM I/O tensors; returns dict name->AP."""
    d = {}

    def inp(name, shape, dt):
        d[name] = nc.dram_tensor(name, list(shape), dt, kind="ExternalInput").ap()

    inp("ea7", (7, p.EC), BF16)
    inp("src16", (128, p.EC // 16), I16)
    inp("xg1T", (128, p.TILES), BF16)
    inp("invcnt", (p.NPC, 1), F32)
    inp("xb1T", (2, p.NPC), F32)
    for li, (ci, co) in enumerate(DIMS, start=1):
        inp(f"wep{li}", (7, ci * co), BF16)
        if li == 1:
            inp(f"rootb{li}", (2, co), F32)
        else:
            inp(f"rootb{li}", (ci, co), F32)
            inp(f"bvec{li}", (1, co), F32)
    inp("onesseg", (128, 1), BF16)
    inp("eyeband", (128, 2 * p.NPC - 1), BF16)
    inp("ones1", (1, p.NPC), F32)
    inp("ident", (128, 128), F32)

    d["cbt"] = nc.dram_tensor("cbt", [p.NPC, p.N], F32, kind="ExternalOutput").ap()
    if p.debug:
        for li, (ci, co) in enumerate(DIMS, start=1):
            d[f"dbg_h{li}"] = nc.dram_tensor(
                f"dbg_h{li}", [p.NPC, co], F32, kind="ExternalOutput"
            ).ap()
            d[f"dbg_U{li}"] = nc.dram_tensor(
                f"dbg_U{li}", [p.NPC, ci * co], F32, kind="ExternalOutput"
            ).ap()
        d["dbg_xg2"] = nc.dram_tensor(
            "dbg_xg2", [128, p.CT, p.GW], BF16, kind="ExternalOutput"
        ).ap()
        d["dbg_T2"] = nc.dram_tensor(
            "dbg_T2", [128, DIMS[1][0] * DIMS[1][1]], BF16, kind="ExternalOutput"
        ).ap()

    # internal DRAM for collectives / gather sources
    for li in (1, 2):
        d[f"agin{li}"] = nc.dram_tensor(f"agin{li}", [p.NPC, p.GW], BF16).ap()
        d[f"agout{li}"] = nc.dram_tensor(
            f"agout{li}", [p.N, p.GW], BF16, addr_space="Shared"
        ).ap()
    d["aginT3"] = nc.dram_tensor("aginT3", [5, p.NPC], F32).ap()
    d["agoutT3"] = nc.dram_tensor(
        "agoutT3", [5 * p.NC, p.NPC], F32, addr_space="Shared"
    ).ap()
    return d


def fsegs(F):
    return [(c, min(c + 512, F)) for c in range(0, F, 512)]


def bcast_ap(ap_in: AP, levels):
    """Build an AP view with given extra [step,count] free levels after the
    partition dim. levels: list of [step_in_elems, count]."""
    part = ap_in.ap[0]
    return bass.AP(ap_in.tensor, ap_in.offset, [list(part)] + [list(x) for x in levels])


def build_kernel(tc: tile.TileContext, io, p: P):
    nc = tc.nc
    NPC, TILES, TPN = p.NPC, p.TILES, p.TPN
    replica = [list(range(p.NC))]

    with (
        tc.tile_pool(name="persist", bufs=1) as pp,
        tc.tile_pool(name="io", bufs=2) as iop,
        tc.tile_pool(name="T", bufs=3) as tp,
        tc.tile_pool(name="w", bufs=3) as wp,
        tc.tile_pool(name="aps", bufs=2, space="PSUM") as ap_pool,
        tc.tile_pool(name="ups", bufs=1, space="PSUM") as up_pool,
        tc.tile_pool(name="sp", bufs=2, space="PSUM") as sp_pool,
    ):
        nc.gpsimd.load_library(library_config.mlp)
        # ---- persistent SBUF: load constants
        ea7 = pp.tile([7, p.EC], BF16, tag="ea7")
        nc.sync.dma_start(out=ea7[:], in_=io["ea7"][:])
        src16 = pp.tile([128, p.EC // 16], I16, tag="src16")
        nc.sync.dma_start(out=src16[:], in_=io["src16"][:])
        xg1T = pp.tile([128, p.TILES], BF16, tag="xg1T")
        nc.sync.dma_start(out=xg1T[:], in_=io["xg1T"][:])
        invcnt = pp.tile([NPC, 1], F32, tag="invcnt")
        nc.sync.dma_start(out=invcnt[:], in_=io["invcnt"][:])
        xb1T = pp.tile([2, NPC], F32, tag="xb1T")
        nc.sync.dma_start(out=xb1T[:], in_=io["xb1T"][:])
        wep = {}
        rootb = {}
        bvec = {}
        for li, (ci, co) in enumerate(DIMS, start=1):
            wep[li] = pp.tile([7, ci * co], BF16, tag=f"wep{li}", name=f"wep{li}")
            nc.sync.dma_start(out=wep[li][:], in_=io[f"wep{li}"][:])
            rb_shape = [2, co] if li == 1 else [ci, co]
            rootb[li] = pp.tile(rb_shape, F32, tag=f"rootb{li}", name=f"rootb{li}")
            nc.sync.dma_start(out=rootb[li][:], in_=io[f"rootb{li}"][:])
            if li > 1:
                bvec[li] = pp.tile([1, co], F32, tag=f"bvec{li}", name=f"bvec{li}")
                nc.sync.dma_start(out=bvec[li][:], in_=io[f"bvec{li}"][:])
        onesseg = pp.tile([128, 1], BF16, tag="onesseg")
        nc.sync.dma_start(out=onesseg[:], in_=io["onesseg"][:])
        eyeband = pp.tile([128, 2 * NPC - 1], BF16, tag="eyeband")
        nc.sync.dma_start(out=eyeband[:], in_=io["eyeband"][:])
        ones1 = pp.tile([1, NPC], F32, tag="ones1")
        nc.sync.dma_start(out=ones1[:], in_=io["ones1"][:])
        ident = pp.tile([128, 128], F32, tag="ident")
        nc.sync.dma_start(out=ident[:], in_=io["ident"][:])

        h_sb = {}      # layer -> [NPC, cout] f32 local nodes

        for li, (ci, co) in enumerate(DIMS, start=1):
            F = ci * co
            G = p.G[li - 1]
            NB = TILES // G

            # ---------- gather x[src] for this layer (li >= 2)
            xg_chunks = []
            if li >= 2:
                for g in range(p.XCH):
                    xgc = iop.tile([128, p.CT, p.GW], BF16, tag="xgc")
                    nidx = p.CT * 128
                    nc.gpsimd.dma_gather(
                        out_ap=xgc[:],
                        in_ap=io[f"agout{li - 1}"][:],
                        idxs_ap=src16[:, g * (nidx // 16):(g + 1) * (nidx // 16)],
                        num_idxs=nidx,
                        num_idxs_reg=nidx,
                        elem_size=p.GW,
                    )
                    xg_chunks.append(xgc)
                    if p.debug and li == 2 and g == 0:
                        nc.sync.dma_start(out=io["dbg_xg2"][:], in_=xgc[:])

            # ---------- main edge loop (groups of G tiles)
            u_ps = up_pool.tile([128, F], F32, tag="u")
            for bi in range(NB):
                a_ps = ap_pool.tile([128, G, F], F32, tag="a")
                for q in range(G):
                    t = bi * G + q
                    for (c0, c1) in fsegs(F):
                        nc.tensor.matmul(
                            a_ps[:, q, c0:c1],
                            lhsT=ea7[:, t * 128:(t + 1) * 128],
                            rhs=wep[li][:, c0:c1],
                            start=True,
                            stop=True,
                        )
                T_sb = tp.tile([128, G, F], BF16, tag="T")
                # ISA mem patterns allow at most 2 free dims, so views below
                # are [P, a, b] only.
                if li == 1:
                    # one fused relu*xg over the whole group:
                    # [128, G, F] with per-(partition,tile) scalar broadcast
                    xgv = bcast_ap(xg1T[:, bi * G], [[1, G], [0, F]])
                    nc.vector.scalar_tensor_tensor(
                        out=T_sb[:], in0=a_ps[:], scalar=0.0, in1=xgv,
                        op0=OP.max, op1=OP.mult,
                    )
                elif li == 2:
                    ch = (bi * G) // p.CT
                    lt = (bi * G) % p.CT
                    xgc = xg_chunks[ch]
                    xgv = bcast_ap(xgc[:, lt, 0], [[0, co], [1, ci]])
                    a_v = a_ps[:, 0, :].rearrange("p (o i) -> p o i", i=ci)
                    t_v = T_sb[:, 0, :].rearrange("p (o i) -> p o i", i=ci)
                    if (bi % 4) != 0:
                        w_sb = wp.tile([128, G, F], BF16, tag="w")
                        nc.scalar.activation(
                            w_sb[:].rearrange("p g f -> p (g f)"),
                            a_ps[:].rearrange("p g f -> p (g f)"),
                            AF.Relu,
                        )
                        w_v = w_sb[:, 0, :].rearrange("p (o i) -> p o i", i=ci)
                        nc.vector.tensor_tensor(
                            out=t_v, in0=w_v, in1=xgv, op=OP.mult
                        )
                    else:
                        nc.vector.scalar_tensor_tensor(
                            out=t_v, in0=a_v, scalar=0.0, in1=xgv,
                            op0=OP.max, op1=OP.mult,
                        )
                else:
                    # l3: ACT relu over the whole group, then per-subtile mult
                    ch = (bi * G) // p.CT
                    lt = (bi * G) % p.CT
                    xgc = xg_chunks[ch]
                    w_sb = wp.tile([128, G, F], BF16, tag="w")
                    nc.scalar.activation(
                        w_sb[:].rearrange("p g f -> p (g f)"),
                        a_ps[:].rearrange("p g f -> p (g f)"),
                        AF.Relu,
                    )
                    for q in range(G):
                        xgv = bcast_ap(xgc[:, lt + q, 0], [[0, co], [1, ci]])
                        nc.vector.tensor_tensor(
                            out=T_sb[:, q, :].rearrange("p (o i) -> p o i", i=ci),
                            in0=w_sb[:, q, :].rearrange("p (o i) -> p o i", i=ci),
                            in1=xgv,
                            op=OP.mult,
                        )
                if p.debug and li == 2 and bi == 0:
                    nc.sync.dma_start(out=io["dbg_T2"][:], in_=T_sb[:, 0, :])
                # segment-sum matmuls: one per 128-edge tile
                for q in range(G):
                    t = bi * G + q
                    j = t // TPN          # local node
                    cg = t % 2            # psum col-group split
                    for (c0, c1) in fsegs(F):
                        nc.tensor.matmul(
                            u_ps[64 * cg:64 * cg + NPC, c0:c1],
                            lhsT=eyeband[:, NPC - 1 - j:2 * NPC - 1 - j],
                            rhs=T_sb[:, q, c0:c1],
                            start=(t < 2),
                            stop=(t >= TILES - 2),
                            tile_position=(0, 64 * cg),
                            skip_group_check=True,
                        )

            # ---------- combine halves, reduce over i, mean, root, relu
            u_hi = wp.tile([NPC, F], F32, tag="uhi")
            nc.vector.tensor_copy(out=u_hi[:], in_=u_ps[64:64 + NPC, :])
            U = wp.tile([NPC, F], F32, tag="U")
            nc.vector.tensor_tensor(
                out=U[:], in0=u_ps[0:NPC, :], in1=u_hi[:], op=OP.add
            )
            s_pre = wp.tile([NPC, co], F32, tag="spre")
            if ci == 1:
                nc.vector.tensor_scalar_mul(s_pre[:], U[:], invcnt[:, 0:1])
            else:
                s_red = wp.tile([NPC, co], F32, tag="sred")
                nc.vector.tensor_reduce(
                    out=s_red[:],
                    in_=U[:].rearrange("p (o i) -> p o i", i=ci),
                    axis=mybir.AxisListType.X,
                    op=OP.add,
                )
                nc.vector.tensor_scalar_mul(s_pre[:], s_red[:], invcnt[:, 0:1])
            # root term
            r_ps = sp_pool.tile([NPC, co], F32, tag="sp")
            if li == 1:
                nc.tensor.matmul(
                    r_ps[:], lhsT=xb1T[:], rhs=rootb[li][:], start=True, stop=True
                )
            else:
                hprev = h_sb[li - 1]
                cprev = DIMS[li - 2][1]
                tr_ps = sp_pool.tile([cprev, NPC], F32, tag="sp")
                nc.tensor.transpose(tr_ps[:], hprev[:, :cprev], ident[0:NPC, 0:NPC])
                hT = wp.tile([cprev, NPC], F32, tag="hT")
                nc.vector.tensor_copy(out=hT[:], in_=tr_ps[:])
                nc.tensor.matmul(
                    r_ps[:], lhsT=hT[:], rhs=rootb[li][:], start=True, stop=False
                )
                nc.tensor.matmul(
                    r_ps[:], lhsT=ones1[:], rhs=bvec[li][:], start=False, stop=True
                )
            h = h_sb.setdefault(
                li, pp.tile([NPC, co], F32, tag=f"h{li}", name=f"h{li}")
            )
            hsum = wp.tile([NPC, co], F32, tag="hsum")
            nc.vector.tensor_tensor(out=hsum[:], in0=s_pre[:], in1=r_ps[:], op=OP.add)
            nc.scalar.activation(h[:], hsum[:], AF.Relu)
            if p.debug:
                nc.sync.dma_start(out=io[f"dbg_h{li}"][:], in_=h[:])
                nc.sync.dma_start(out=io[f"dbg_U{li}"][:], in_=U[:])

            # ---------- publish h
            if li < 3:
                hbf = iop.tile([NPC, p.GW], BF16, tag="hbf")
                nc.vector.memset(hbf[:], 0.0)
                nc.vector.tensor_copy(out=hbf[:, 0:co], in_=h[:])
                nc.sync.dma_start(out=io[f"agin{li}"][:], in_=hbf[:])
                nc.gpsimd.collective_compute(
                    "AllGather",
                    OP.bypass,
                    replica_groups=replica,
                    ins=[io[f"agin{li}"][:]],
                    outs=[io[f"agout{li}"][:]],
                )
            else:
                tr3 = sp_pool.tile([co, NPC], F32, tag="sp")
                nc.tensor.transpose(tr3[:], h[:, :co], ident[0:NPC, 0:NPC])
                h3T = pp.tile([co, NPC], F32, tag="h3T")
                nc.vector.tensor_copy(out=h3T[:], in_=tr3[:])
                nc.sync.dma_start(out=io["aginT3"][:], in_=h3T[:])
                nc.gpsimd.collective_compute(
                    "AllGather",
                    OP.bypass,
                    replica_groups=replica,
                    ins=[io["aginT3"][:]],
                    outs=[io["agoutT3"][:]],
                )

        # -------------------------------------------------- CBT phase
        agT = pp.tile([1, 5 * p.NC * NPC], F32, tag="agT")
        nc.sync.dma_start(
            out=agT[:], in_=io["agoutT3"][:].rearrange("(o a) b -> o (a b)", o=1)
        )
        h3 = h_sb[3]
        for b in range(p.NC):
            repl = sp_pool.tile([NPC, 5, NPC], F32, tag="sp")
            for k in range(5):
                nc.tensor.matmul(
                    repl[:, k, :],
                    lhsT=ones1[:],
                    rhs=agT[0:1, (5 * b + k) * NPC:(5 * b + k + 1) * NPC],
                    start=True,
                    stop=True,
                )
            diff = wp.tile([NPC, NPC, 5], F32, tag="diff")
            h3v = bcast_ap(h3[:, 0], [[0, NPC], [1, 5]])
            rv = repl[:].rearrange("p k j -> p j k")
            nc.vector.tensor_tensor(out=diff[:], in0=h3v, in1=rv, op=OP.subtract)
            cb = wp.tile([NPC, NPC], F32, tag="cb")
            nc.vector.tensor_reduce(
                out=cb[:],
                in_=diff[:],
                axis=mybir.AxisListType.X,
                op=OP.add,
                apply_absolute_value=True,
            )
            nc.sync.dma_start(
                out=io["cbt"][:, b * NPC:(b + 1) * NPC], in_=cb[:]
            )


def build_program(p: P):
    nc = bacc.Bacc(
        "TRN2",
        target_bir_lowering=False,
        debug=False,
        num_devices=p.NC,
    )
    io = declare_io(nc, p)
    with tile.TileContext(nc) as tc:
        build_kernel(tc, io, p)
    nc.compile()
    return nc, io


# ----------------------------------------------------------------- numpy ref


def numpy_reference(x, edge_attr, edge_index, weights):
    src, dst = edge_index[0].astype(np.int64), edge_index[1].astype(np.int64)
    N = x.shape[0]
    h = x.astype(np.float64)
    ea = edge_attr.astype(np.float64)
    for li, (ci, co) in enumerate(DIMS, start=1):
        We, be = weights[f"We{li}"], weights[f"be{li}"]
        root, b = weights[f"root{li}"], weights[f"b{li}"]
        w = np.maximum(ea @ We + be, 0.0).reshape(-1, ci, co)
        msg = np.einsum("ei,eio->eo", h[src], w)
        s = np.zeros((N, co))
        np.add.at(s, dst, msg)
        cnt = np.zeros((N, 1))
        np.add.at(cnt, dst, 1.0)
        agg = s / np.maximum(cnt, 1.0)
        h = np.maximum(agg + h @ root + b, 0.0)
    return np.abs(h[:, None, :] - h[None, :, :]).sum(axis=2)


# ----------------------------------------------------------------- runner


def run(x, edge_attr, edge_index, weights, p: P, trace=False):
    from concourse.bass_utils import run_bass_kernel_spmd

    in_maps = host_prep(x, edge_attr, edge_index, weights, p)
    nc, io = build_program(p)
    res = run_bass_kernel_spmd(
        nc, in_maps, list(range(p.NC)), trace=trace
    )
    cbt = np.concatenate([r["cbt"] for r in res.results], axis=0)
    return cbt, res


# ----------------------------------------------------------- harness entry


def kernel(**inputs) -> np.ndarray:
    """Full-input entry: shards edges/nodes across 8 NeuronCores, runs the
    Bass kernel, returns the [N, N] CBT matrix."""
    x = np.asarray(inputs["x"], dtype=np.float32)
    ea = np.asarray(inputs["edge_attr"], dtype=np.float32)
    ei = np.asarray(inputs["edge_index"])
    weights = {
        k: np.asarray(v)
        for k, v in inputs.items()
        if k not in ("x", "edge_attr", "edge_index")
    }
    N = x.shape[0]
    dst = ei[1].astype(np.int64)
    maxdeg = int(np.bincount(dst, minlength=N).max())
    TPN = max(1, -(-maxdeg // 128))
    p = P(N=N, n_cores=8, TPN=TPN)
    cbt, _res = run(x, ea, ei, weights, p, trace=False)
    return cbt.astype(np.float32)
